# revision 16
# baseline (speedup 1.0000x reference)
"""Trainium2 Bass kernel for nn_HGraphAttentionLayer (GAT-style layer, 8 NeuronCores).

Math (reference):
  feats[h,n,o]  = concat(input[:5000] @ proj_rna[h], input[5000:] @ proj_dis[h])
  s_src[h,n]    = feats[h,n,:] @ score_src[h];  s_tgt likewise
  attn[h,i,j]   = softmax_over_i( mask[i,j] + leaky_relu(s_src[h,i]+s_tgt[h,j], 0.2) )
  vals[i,o]     = mean_h( sum_j attn[h,i,j] * feats[h,j,o] )
  out           = elu( instancenorm(vals) + input @ residual_w.T )

Sharding: each core owns N/8 = 1024 query rows (i). Softmax reduces over i,
so partial column sums d[h,j] are AllGathered per j-stripe.

Key design (v2):
 - mask passed from host as a pre-transposed {0,1} bf16 indicator M^T[j,i];
   e = M * exp(lrelu(a_i+b_j)) so the mask enters via one 2x-mode
   tensor_tensor_reduce (which also produces d via accum) instead of a
   1x-mode 3-input add chain.
 - z = a+b is a 4x-mode tensor_scalar (per-partition scalar b_j).
 - lrelu runs on ACT (Prelu, batched [128,4096] over 4 heads) for most
   j-tiles and on DVE (2-op mul/max) for a tunable fraction, balancing
   the two engines. exp is one batched ACT op per j-tile.
 - feats computed just-in-time per stripe from SBUF-resident inputT
   (no DRAM spills); input/proj/residual weights pre-cast + pre-transposed
   on host.
 - g = feats/d runs on the otherwise-idle GpSimd engine.
"""
import numpy as np

N, F, H, O = 8192, 256, 4, 128
N_CORES = 8
MY_N = N // N_CORES          # 1024 rows per core
N_RNA = 5000
SLOPE = 0.2
EPS = 1e-5
N_STRIPES = 8
JT = 8                       # j-tiles (128 j each) per stripe
NCH = N // 128               # 64 node chunks
FC = F // 128                # 2 f chunks
SPLIT_CH = N_RNA // 128      # chunk 39 contains the rna/dis boundary
SPLIT_ROW = N_RNA - SPLIT_CH * 128  # row 8 within chunk 39
HQ = MY_N                    # head-quadrant stride in the batched tiles

# j-tile slots (s*8+jt) where lrelu runs on DVE instead of ACT (balance knob)
DVE_LRELU = {i for i in range(64) if i % 8 == 3}  # 8 of 64 slots


def em_on_gp(slot, h):
    """Route the masked multiply for (slot, h) to GpSimd (balance knob)."""
    return h == 0 or (h == 1 and slot % 2 == 0)

_cached = {}


def _build():
    import concourse.bass as bass
    import concourse.bacc as bacc
    import concourse.mybir as mybir
    import concourse.tile as tile

    f32 = mybir.dt.float32
    bf16 = mybir.dt.bfloat16
    Alu = mybir.AluOpType
    Act = mybir.ActivationFunctionType

    nc = bacc.Bacc("TRN2", target_bir_lowering=False, debug=False,
                   enable_asserts=False, num_devices=N_CORES)

    # ---- I/O -----------------------------------------------------------
    maskT = nc.dram_tensor("maskT", [N, MY_N], bf16, kind="ExternalInput").ap()
    inputT_in = nc.dram_tensor("inputT", [FC, 128, N], bf16, kind="ExternalInput").ap()
    rowsT_rna_in = nc.dram_tensor("rowsT_rna", [FC, 128, MY_N], bf16,
                                  kind="ExternalInput").ap()
    rowsT_dis_in = nc.dram_tensor("rowsT_dis", [FC, 128, MY_N], bf16,
                                  kind="ExternalInput").ap()
    projcat_in = nc.dram_tensor("projcat", [2, FC, 128, H * O], bf16,
                                kind="ExternalInput").ap()
    scores_in = nc.dram_tensor("scores", [8, 128], f32, kind="ExternalInput").ap()
    wrT_in = nc.dram_tensor("wrT", [FC, 128, 128], bf16, kind="ExternalInput").ap()
    identf_in = nc.dram_tensor("identf", [128, 128], f32, kind="ExternalInput").ap()
    sel39_in = nc.dram_tensor("sel39", [128, 1], f32, kind="ExternalInput").ap()
    invsel39_in = nc.dram_tensor("invsel39", [128, 1], f32, kind="ExternalInput").ap()
    out_dram = nc.dram_tensor("out", [O, MY_N], f32, kind="ExternalOutput").ap()

    RG = [list(range(N_CORES))]

    with tile.TileContext(nc) as tc:
        with (
            tc.tile_pool(name="const", bufs=1) as constp,
            tc.tile_pool(name="ps_work", bufs=2, space="PSUM") as ps_work,
            tc.tile_pool(name="ps_s", bufs=2, space="PSUM") as ps_s,
            tc.tile_pool(name="ps_f", bufs=2, space="PSUM") as ps_f,
            tc.tile_pool(name="ps_vals", bufs=1, space="PSUM") as ps_vals,
            tc.tile_pool(name="dram", bufs=1, space="DRAM") as dram,
        ):
            pro = tc.alloc_tile_pool(name="pro", bufs=3)
            # ---- DRAM scratch ------------------------------------------
            d_in = [dram.tile([128, 32], f32, tag=f"din{s}", name=f"din{s}")
                    for s in range(N_STRIPES)]
            d_out = [dram.tile([128 * N_CORES, 32], f32, tag=f"dout{s}",
                               name=f"dout{s}") for s in range(N_STRIPES)]
            st_in = dram.tile([1, 32], f32, tag="stin", name="stin")
            st_out = dram.tile([1, 32], f32, tag="stout", name="stout")
            dum_in = dram.tile([1, 16], f32, tag="dumin", name="dumin")
            dum_out = dram.tile([1, 16], f32, tag="dumout", name="dumout")
            arow_dram = dram.tile([H, MY_N], f32, tag="arowd", name="arowd")

            # ---- constants ---------------------------------------------
            identf = constp.tile([128, 128], f32, tag="identf", name="identf")
            nc.sync.dma_start(identf[:], identf_in)
            ones_col = constp.tile([128, 1], f32, tag="ones_col", name="ones_col")
            nc.vector.memset(ones_col[:], 1.0)
            ones_row = constp.tile([1, 512], f32, tag="ones_row", name="ones_row")
            nc.vector.memset(ones_row[:], 1.0)
            sel39 = constp.tile([128, 1], f32, tag="sel39", name="sel39")
            nc.sync.dma_start(sel39[:], sel39_in)
            invsel39 = constp.tile([128, 1], f32, tag="invsel39", name="invsel39")
            nc.sync.dma_start(invsel39[:], invsel39_in)

            # warm up the collective stack early
            zr = constp.tile([1, 16], f32, tag="zr", name="zr")
            nc.vector.memset(zr[:], 0.0)
            nc.sync.dma_start(dum_in[:], zr[:])
            nc.gpsimd.collective_compute(
                "AllReduce", Alu.add, replica_groups=RG,
                ins=[dum_in.opt()], outs=[dum_out.opt()])

            # ---- resident inputs ---------------------------------------
            inputT = [constp.tile([128, N], bf16, tag=f"inT{fc}", name=f"inT{fc}")
                      for fc in range(FC)]
            for fc in range(FC):
                nc.sync.dma_start(inputT[fc][:], inputT_in[fc])
            rnaT = [constp.tile([128, MY_N], bf16, tag=f"rnaT{fc}", name=f"rnaT{fc}")
                    for fc in range(FC)]
            disT = [constp.tile([128, MY_N], bf16, tag=f"disT{fc}", name=f"disT{fc}")
                    for fc in range(FC)]
            rowsT = [constp.tile([128, MY_N], bf16, tag=f"rowsT{fc}", name=f"rowsT{fc}")
                     for fc in range(FC)]
            for fc in range(FC):
                nc.sync.dma_start(rnaT[fc][:], rowsT_rna_in[fc])
                nc.sync.dma_start(disT[fc][:], rowsT_dis_in[fc])
                nc.vector.tensor_add(rowsT[fc][:], rnaT[fc][:], disT[fc][:])

            wrT = [constp.tile([128, 128], bf16, tag=f"wrT{fc}", name=f"wrT{fc}")
                   for fc in range(FC)]
            for fc in range(FC):
                nc.sync.dma_start(wrT[fc][:], wrT_in[fc])
            projc = {}
            for t in range(2):
                for fc in range(FC):
                    pt = constp.tile([128, H * O], bf16, tag=f"pj{t}{fc}",
                                     name=f"pj{t}{fc}")
                    nc.sync.dma_start(pt[:], projcat_in[t, fc])
                    projc[(t, fc)] = pt

            # ---- q_rhs[t,fc] = [128f, 8] (cols 0-3 src h, 4-7 tgt h) ----
            q_rhs = {(t, fc): constp.tile([128, 8], bf16, tag=f"q{t}{fc}",
                                          name=f"q{t}{fc}")
                     for t in range(2) for fc in range(FC)}
            for si in range(2):
                for h in range(H):
                    srow0 = pro.tile([1, 128], f32, tag="srow0", name="srow0",
                                     bufs=2)
                    nc.sync.dma_start(srow0[:], scores_in[si * 4 + h:si * 4 + h + 1, :])
                    wb = pro.tile([128, 128], f32, tag="wb", name="wb", bufs=2)
                    nc.gpsimd.partition_broadcast(wb[:], srow0[:])
                    for t in range(2):
                        for fc in range(FC):
                            qcol = pro.tile([128, 1], f32, tag="qcol", name="qcol",
                                            bufs=2)
                            qscr = pro.tile([128, O], f32, tag="qscr", name="qscr",
                                            bufs=2)
                            nc.vector.scalar_tensor_tensor(
                                qscr[:], projc[(t, fc)][:, h * O:(h + 1) * O], 1.0,
                                wb[:], op0=Alu.mult, op1=Alu.mult,
                                accum_out=qcol[:])
                            nc.vector.tensor_copy(
                                q_rhs[(t, fc)][:, si * 4 + h:si * 4 + h + 1],
                                qcol[:])

            # ---- s for all chunks: s_big [128, 64*8] f32 ----------------
            def chunk_types(ch):
                if ch < SPLIT_CH:
                    return [0]
                if ch > SPLIT_CH:
                    return [1]
                return [0, 1]

            s_big = constp.tile([128, NCH * 8], f32, tag="sbig", name="sbig")
            for ch in range(NCH):
                tps = chunk_types(ch)
                res = {}
                for t in tps:
                    ps_sc = ps_s.tile([128, 8], f32, tag="small", name="pssc")
                    for fc in range(FC):
                        nc.tensor.matmul(
                            ps_sc[:], inputT[fc][:, ch * 128:(ch + 1) * 128],
                            q_rhs[(t, fc)], start=(fc == 0), stop=(fc == FC - 1))
                    if len(tps) == 1:
                        nc.vector.tensor_copy(s_big[:, ch * 8:ch * 8 + 8], ps_sc[:])
                    else:
                        tmp = pro.tile([128, 8], f32, tag="stmp", name="stmp",
                                       bufs=3)
                        nc.vector.tensor_copy(tmp[:], ps_sc[:])
                        res[t] = tmp
                if len(tps) == 2:
                    t1 = pro.tile([128, 8], f32, tag="sbl", name="sbl", bufs=2)
                    nc.vector.tensor_scalar_mul(t1[:], res[1][:], invsel39[:])
                    nc.vector.scalar_tensor_tensor(
                        s_big[:, ch * 8:ch * 8 + 8], res[0][:], sel39[:], t1[:],
                        op0=Alu.mult, op1=Alu.add)

            # ---- s_src for my rows -> A_bcast[h] [128, MY_N] bf16 -------
            for ic in range(MY_N // 128):
                ps_sr = ps_s.tile([128, 8], f32, tag="small", name="pssr")
                k = 0
                for Tt in (rnaT, disT):
                    for fc in range(FC):
                        nc.tensor.matmul(ps_sr[:], Tt[fc][:, ic * 128:(ic + 1) * 128],
                                         q_rhs[(0 if Tt is rnaT else 1, fc)],
                                         start=(k == 0), stop=(k == 3))
                        k += 1
                srow = pro.tile([128, 8], f32, tag="srow", name="srow", bufs=2)
                nc.vector.tensor_copy(srow[:], ps_sr[:])
                tpsm = ps_work.tile([128, 128], f32, tag="tp", name="tps")
                nc.tensor.transpose(tpsm[0:8, :], srow[:], identf[:])
                srT = pro.tile([8, 128], f32, tag="srT", name="srT", bufs=2)
                nc.vector.tensor_copy(srT[:], tpsm[0:8, :])
                for h in range(H):
                    nc.sync.dma_start(arow_dram[h, ic * 128:(ic + 1) * 128],
                                      srT[h:h + 1, :])
            A_bcast = []
            for h in range(H):
                af = pro.tile([128, MY_N], f32, tag="af", name="af", bufs=2)
                nc.sync.dma_start(af[:], arow_dram[h:h + 1, :].partition_broadcast(128))
                ab = constp.tile([128, MY_N], bf16, tag=f"ab{h}", name=f"ab{h}")
                nc.vector.tensor_copy(ab[:], af[:])
                A_bcast.append(ab)

            # ---- main loop over j-stripes ------------------------------
            pro.release()
            mtp = tc.alloc_tile_pool(name="mtp", bufs=10)
            zp = tc.alloc_tile_pool(name="zp", bufs=2)
            yp = tc.alloc_tile_pool(name="yp", bufs=2)
            ep = tc.alloc_tile_pool(name="ep", bufs=8)
            fsp = tc.alloc_tile_pool(name="fsp", bufs=4)
            gp4 = tc.alloc_tile_pool(name="gp4", bufs=3)
            dpool = tc.alloc_tile_pool(name="dpool", bufs=3)
            vals_ps = ps_vals.tile([128, MY_N], f32, tag="big", name="vals")

            for s in range(N_STRIPES):
                # transposed 0/1 mask tiles for this stripe (plain loads)
                mts = []
                for jt in range(JT):
                    mt = mtp.tile([128, MY_N], bf16, tag="mt", name="mt")
                    nc.sync.dma_start(
                        mt[:], maskT[(s * JT + jt) * 128:(s * JT + jt + 1) * 128, :])
                    mts.append(mt)

                # feats for this stripe's chunks (JIT from resident inputT)
                fsb = []
                for jt in range(JT):
                    ch = s * JT + jt
                    tps = chunk_types(ch)
                    res = {}
                    for t in tps:
                        psf = ps_f.tile([128, H * O], f32, tag="psf", name="psf")
                        for fc in range(FC):
                            nc.tensor.matmul(psf[:], inputT[fc][:, ch * 128:(ch + 1) * 128],
                                             projc[(t, fc)][:],
                                             start=(fc == 0), stop=(fc == FC - 1))
                        if len(tps) == 1:
                            fs = fsp.tile([128, H * O], bf16, tag="fs", name="fs")
                            nc.vector.tensor_copy(fs[:], psf[:])
                        else:
                            tmp = fsp.tile([128, H * O], bf16, tag="fbl", name="fbl",
                                           bufs=2)
                            nc.vector.tensor_copy(tmp[:], psf[:])
                            res[t] = tmp
                    if len(tps) == 2:
                        fs = fsp.tile([128, H * O], bf16, tag="fs", name="fs")
                        t1 = fsp.tile([128, H * O], bf16, tag="fbl2", name="fbl2",
                                      bufs=2)
                        nc.vector.tensor_scalar_mul(t1[:], res[1][:], invsel39[:])
                        nc.vector.scalar_tensor_tensor(
                            fs[:], res[0][:], sel39[:], t1[:],
                            op0=Alu.mult, op1=Alu.add)
                    fsb.append(fs)

                # elementwise chain per j-tile (4 heads batched in free dim)
                d_all = dpool.tile([128, 32], f32, tag="dall", name="dall")
                ebs = []
                for jt in range(JT):
                    ch = s * JT + jt
                    zb = zp.tile([128, H * HQ], bf16, tag="zb", name="zb")
                    for h in range(H):
                        nc.vector.tensor_scalar_add(
                            zb[:, h * HQ:(h + 1) * HQ], A_bcast[h][:],
                            s_big[:, ch * 8 + 4 + h:ch * 8 + 4 + h + 1])
                    yb = yp.tile([128, H * HQ], bf16, tag="yb", name="yb")
                    if (s * JT + jt) in DVE_LRELU:
                        for h in range(H):
                            sl = slice(h * HQ, (h + 1) * HQ)
                            nc.vector.tensor_scalar_mul(yb[:, sl], zb[:, sl], SLOPE)
                            nc.vector.tensor_tensor(yb[:, sl], yb[:, sl], zb[:, sl],
                                                    op=Alu.max)
                    else:
                        nc.scalar.activation(yb[:], zb[:], Act.Prelu, alpha=SLOPE)
                    eb = ep.tile([128, H * HQ], bf16, tag="eb", name="eb")
                    nc.scalar.activation(eb[:], yb[:], Act.Exp)
                    for h in range(H):
                        sl = slice(h * HQ, (h + 1) * HQ)
                        eng = nc.gpsimd if em_on_gp(s * JT + jt, h) else nc.vector
                        eng.tensor_tensor(eb[:, sl], eb[:, sl], mts[jt][:],
                                          op=Alu.mult)
                        nc.vector.tensor_scalar(
                            zb[:, sl], eb[:, sl], 1.0, 0.0, op0=Alu.mult,
                            op1=Alu.add,
                            accum_out=d_all[:, h * 8 + jt:h * 8 + jt + 1])
                    ebs.append(eb)

                # complete d across cores (partial sums over i-rows)
                nc.sync.dma_start(d_in[s][:], d_all[:])
                nc.gpsimd.collective_compute(
                    "AllGather", Alu.bypass, replica_groups=RG,
                    ins=[d_in[s].opt()], outs=[d_out[s].opt()])
                dg = dpool.tile([128, 256], f32, tag="dg", name="dg")
                for r in range(N_CORES):
                    nc.sync.dma_start(dg[:, r * 32:(r + 1) * 32],
                                      d_out[s][r * 128:(r + 1) * 128, :])
                d_sum = dpool.tile([128, 32], f32, tag="dsum", name="dsum")
                nc.vector.tensor_add(d_sum[:], dg[:, 0:32], dg[:, 32:64])
                for r in range(2, N_CORES):
                    nc.vector.tensor_add(d_sum[:], d_sum[:], dg[:, r * 32:(r + 1) * 32])
                dinv = dpool.tile([128, 32], f32, tag="dinv", name="dinv")
                nc.vector.reciprocal(dinv[:], d_sum[:])

                # g = feats/d (GpSimd) ; vals^T += g^T-contract-e
                for jt in range(JT):
                    g4 = gp4.tile([128, H * O], bf16, tag="g4", name="g4")
                    for h in range(H):
                        nc.gpsimd.tensor_scalar_mul(
                            g4[:, h * O:(h + 1) * O], fsb[jt][:, h * O:(h + 1) * O],
                            dinv[:, h * 8 + jt:h * 8 + jt + 1])
                    eb = ebs[jt]
                    for h in range(H):
                        first = (s == 0) and jt == 0 and h == 0
                        last = (s == N_STRIPES - 1) and jt == JT - 1 and h == H - 1
                        nc.tensor.matmul(vals_ps[:, 0:512], g4[:, h * O:(h + 1) * O],
                                         eb[:, h * HQ:h * HQ + 512],
                                         start=first, stop=last)
                        nc.tensor.matmul(vals_ps[:, 512:1024], g4[:, h * O:(h + 1) * O],
                                         eb[:, h * HQ + 512:(h + 1) * HQ],
                                         start=first, stop=last)

            # ---- tail: instance norm + residual + elu ------------------
            dpool.release()
            gp4.release()
            fsp.release()
            ep.release()
            yp.release()
            zp.release()
            mtp.release()
            tailp = tc.alloc_tile_pool(name="tail", bufs=1)
            vs = tailp.tile([128, MY_N], f32, tag="vs", name="vs")
            srow1 = tailp.tile([128, 1], f32, tag="srow1", name="srow1")
            nc.scalar.activation(vs[:], vals_ps[:], Act.Copy, scale=0.25,
                                 accum_out=srow1[:])
            vsq = tailp.tile([128, MY_N], f32, tag="vsq", name="vsq")
            srow2 = tailp.tile([128, 1], f32, tag="srow2", name="srow2")
            nc.scalar.activation(vsq[:], vs[:], Act.Square, accum_out=srow2[:])

            ps1 = ps_s.tile([1, 1], f32, tag="small", name="ps1")
            nc.tensor.matmul(ps1[:], srow1[:], ones_col[:])
            ps2 = ps_s.tile([1, 1], f32, tag="small", name="ps2")
            nc.tensor.matmul(ps2[:], srow2[:], ones_col[:])
            stv = tailp.tile([1, 32], f32, tag="stv", name="stv")
            nc.vector.memset(stv[:], 0.0)
            nc.vector.tensor_copy(stv[0:1, 0:1], ps1[:])
            nc.vector.tensor_copy(stv[0:1, 16:17], ps2[:])
            nc.sync.dma_start(st_in[:], stv[:])
            nc.gpsimd.collective_compute(
                "AllReduce", Alu.add, replica_groups=RG,
                ins=[st_in.opt()], outs=[st_out.opt()])
            str_ = tailp.tile([1, 32], f32, tag="str", name="str")
            nc.sync.dma_start(str_[:], st_out[:])

            c = 1.0 / float(N * O)
            mu = tailp.tile([1, 1], f32, tag="mu", name="mu")
            nc.vector.tensor_scalar_mul(mu[:], str_[0:1, 0:1], c)
            m2 = tailp.tile([1, 1], f32, tag="m2", name="m2")
            nc.vector.tensor_scalar_mul(m2[:], str_[0:1, 16:17], c)
            mu2 = tailp.tile([1, 1], f32, tag="mu2", name="mu2")
            nc.vector.tensor_mul(mu2[:], mu[:], mu[:])
            var = tailp.tile([1, 1], f32, tag="var", name="var")
            nc.vector.tensor_sub(var[:], m2[:], mu2[:])
            vpe = tailp.tile([1, 1], f32, tag="vpe", name="vpe")
            nc.vector.tensor_scalar_add(vpe[:], var[:], EPS)
            sd = tailp.tile([1, 1], f32, tag="sd", name="sd")
            nc.scalar.activation(sd[:], vpe[:], Act.Sqrt)
            rstd = tailp.tile([1, 1], f32, tag="rstd", name="rstd")
            nc.vector.reciprocal(rstd[:], sd[:])
            negmurs = tailp.tile([1, 1], f32, tag="negmurs", name="negmurs")
            nc.vector.tensor_mul(negmurs[:], mu[:], rstd[:])
            nc.vector.tensor_scalar_mul(negmurs[:], negmurs[:], -1.0)

            a_col = tailp.tile([128, 1], f32, tag="acol", name="acol")
            nc.gpsimd.partition_broadcast(a_col[:], rstd[:])
            b_row = tailp.tile([1, 128], f32, tag="brow", name="brow")
            nc.scalar.activation(b_row[:], ones_row[0:1, 0:128], Act.Copy,
                                 scale=negmurs[:])

            r_ps = ps_vals.tile([128, MY_N], f32, tag="big", name="resid")
            for half in range(2):
                sl = slice(half * 512, (half + 1) * 512)
                for fc in range(FC):
                    nc.tensor.matmul(r_ps[:, sl], wrT[fc][:], rowsT[fc][:, sl],
                                     start=(fc == 0), stop=False)
                nc.tensor.matmul(r_ps[:, sl], b_row[:], ones_row[:],
                                 start=False, stop=True)

            pre = tailp.tile([128, MY_N], f32, tag="pre", name="pre")
            nc.vector.scalar_tensor_tensor(pre[:], vs[:], a_col[:], r_ps[:],
                                           op0=Alu.mult, op1=Alu.add)
            negp = tailp.tile([128, MY_N], f32, tag="negp", name="negp")
            nc.vector.tensor_scalar_min(negp[:], pre[:], 0.0)
            w = tailp.tile([128, MY_N], f32, tag="w", name="w")
            nc.scalar.activation(w[:], negp[:], Act.Exp)
            r1 = tailp.tile([128, MY_N], f32, tag="r1", name="r1")
            nc.vector.tensor_scalar_max(r1[:], pre[:], 0.0)
            outt = tailp.tile([128, MY_N], f32, tag="outt", name="outt")
            nc.vector.scalar_tensor_tensor(outt[:], w[:], -1.0, r1[:],
                                           op0=Alu.add, op1=Alu.add)
            nc.sync.dma_start(out_dram, outt[:])
            tailp.release()

    nc.compile()
    return nc


def _get_nc():
    if "nc" not in _cached:
        _cached["nc"] = _build()
    return _cached["nc"]


def kernel(input_mat, connectivity_mask, proj_rna, proj_dis, score_src,
           score_tgt, residual_w):
    from concourse.bass_utils import run_bass_kernel_spmd
    from ml_dtypes import bfloat16

    nc = _get_nc()
    input_mat = np.asarray(input_mat, np.float32)
    connectivity_mask = np.asarray(connectivity_mask, np.float32)
    proj_rna = np.asarray(proj_rna, np.float32)
    proj_dis = np.asarray(proj_dis, np.float32)
    score_src = np.asarray(score_src, np.float32)
    score_tgt = np.asarray(score_tgt, np.float32)
    residual_w = np.asarray(residual_w, np.float32)

    ident = np.eye(128, dtype=np.float32)
    sel39 = (np.arange(128) < SPLIT_ROW).astype(np.float32)[:, None]

    # shared (replicated) prepped tensors
    inputT_b = np.ascontiguousarray(input_mat.T.astype(bfloat16)).reshape(
        FC, 128, N)
    projcat = np.empty((2, FC, 128, H * O), np.float32)
    for t, pj in enumerate((proj_rna, proj_dis)):
        for fc in range(FC):
            for h in range(H):
                projcat[t, fc, :, h * O:(h + 1) * O] = pj[h, fc * 128:(fc + 1) * 128, :]
    projcat_b = projcat.astype(bfloat16)
    scores = np.concatenate([score_src[:, :, 0], score_tgt[:, :, 0]],
                            axis=0).astype(np.float32)  # [8, 128]
    wrT = np.empty((FC, 128, 128), np.float32)
    for fc in range(FC):
        wrT[fc] = residual_w[:, fc * 128:(fc + 1) * 128].T
    wrT_b = wrT.astype(bfloat16)

    rna_mask = (np.arange(N) < N_RNA).astype(np.float32)[:, None]
    in_rna_full = input_mat * rna_mask
    in_dis_full = input_mat * (1.0 - rna_mask)

    in_maps = []
    for k in range(N_CORES):
        r0, r1 = k * MY_N, (k + 1) * MY_N
        maskT_k = np.ascontiguousarray(
            (connectivity_mask[r0:r1, :] == 0.0).astype(bfloat16).T)
        rowsT_rna = np.ascontiguousarray(
            in_rna_full[r0:r1].T.astype(bfloat16)).reshape(FC, 128, MY_N)
        rowsT_dis = np.ascontiguousarray(
            in_dis_full[r0:r1].T.astype(bfloat16)).reshape(FC, 128, MY_N)
        in_maps.append({
            "maskT": maskT_k,
            "inputT": inputT_b,
            "rowsT_rna": rowsT_rna,
            "rowsT_dis": rowsT_dis,
            "projcat": projcat_b,
            "scores": scores,
            "wrT": wrT_b,
            "identf": ident,
            "sel39": sel39,
            "invsel39": 1.0 - sel39,
        })

    res = run_bass_kernel_spmd(nc, in_maps, core_ids=list(range(N_CORES)))
    _cached["last_result"] = res
    out = np.empty((N, O), np.float32)
    for k in range(N_CORES):
        out[k * MY_N:(k + 1) * MY_N, :] = res.results[k]["out"].T
    return out


# revision 21
# speedup vs baseline: 2.0590x; 2.0590x over previous
"""Trainium2 Bass kernel for nn_HGraphAttentionLayer (GAT-style layer, 8 NeuronCores).

Math (reference):
  feats[h,n,o]  = concat(input[:5000] @ proj_rna[h], input[5000:] @ proj_dis[h])
  s_src[h,n]    = feats[h,n,:] @ score_src[h];  s_tgt likewise
  attn[h,i,j]   = softmax_over_i( mask[i,j] + leaky_relu(s_src[h,i]+s_tgt[h,j], 0.2) )
  vals[i,o]     = mean_h( sum_j attn[h,i,j] * feats[h,j,o] )
  out           = elu( instancenorm(vals) + input @ residual_w.T )

Sharding: each core owns N/8 = 1024 query rows (i). Softmax reduces over i,
so partial column sums d[h,j] are AllGathered per j-stripe.

Key design (v2):
 - mask passed from host as a pre-transposed {0,1} bf16 indicator M^T[j,i];
   e = M * exp(lrelu(a_i+b_j)) so the mask enters via one 2x-mode
   tensor_tensor_reduce (which also produces d via accum) instead of a
   1x-mode 3-input add chain.
 - z = a+b is a 4x-mode tensor_scalar (per-partition scalar b_j).
 - lrelu runs on ACT (Prelu, batched [128,4096] over 4 heads) for most
   j-tiles and on DVE (2-op mul/max) for a tunable fraction, balancing
   the two engines. exp is one batched ACT op per j-tile.
 - feats computed just-in-time per stripe from SBUF-resident inputT
   (no DRAM spills); input/proj/residual weights pre-cast + pre-transposed
   on host.
 - g = feats/d runs on the otherwise-idle GpSimd engine.
"""
import numpy as np

N, F, H, O = 8192, 256, 4, 128
N_CORES = 8
MY_N = N // N_CORES          # 1024 rows per core
N_RNA = 5000
SLOPE = 0.2
EPS = 1e-5
N_STRIPES = 8
JT = 8                       # j-tiles (128 j each) per stripe
NCH = N // 128               # 64 node chunks
FC = F // 128                # 2 f chunks
SPLIT_CH = N_RNA // 128      # chunk 39 contains the rna/dis boundary
SPLIT_ROW = N_RNA - SPLIT_CH * 128  # row 8 within chunk 39
HQ = MY_N                    # head-quadrant stride in the batched tiles

# j-tile slots (s*8+jt) where lrelu runs on DVE instead of ACT (balance knob)
DVE_LRELU = {i for i in range(64) if i % 4 == 1}  # 16 of 64 slots




_cached = {}


def _build():
    import concourse.bass as bass
    import concourse.bacc as bacc
    import concourse.mybir as mybir
    import concourse.tile as tile

    f32 = mybir.dt.float32
    bf16 = mybir.dt.bfloat16
    Alu = mybir.AluOpType
    Act = mybir.ActivationFunctionType

    nc = bacc.Bacc("TRN2", target_bir_lowering=False, debug=False,
                   enable_asserts=False, num_devices=N_CORES)

    # ---- I/O -----------------------------------------------------------
    maskT = nc.dram_tensor("maskT", [N, MY_N], bf16, kind="ExternalInput").ap()
    inputT_in = nc.dram_tensor("inputT", [FC, 128, N], bf16, kind="ExternalInput").ap()
    rowsT_rna_in = nc.dram_tensor("rowsT_rna", [FC, 128, MY_N], bf16,
                                  kind="ExternalInput").ap()
    rowsT_dis_in = nc.dram_tensor("rowsT_dis", [FC, 128, MY_N], bf16,
                                  kind="ExternalInput").ap()
    projcat_in = nc.dram_tensor("projcat", [2, FC, 128, H * O], bf16,
                                kind="ExternalInput").ap()
    scores_in = nc.dram_tensor("scores", [8, 128], f32, kind="ExternalInput").ap()
    wrT_in = nc.dram_tensor("wrT", [FC, 128, 128], bf16, kind="ExternalInput").ap()
    identf_in = nc.dram_tensor("identf", [128, 128], f32, kind="ExternalInput").ap()
    sel39_in = nc.dram_tensor("sel39", [128, 1], f32, kind="ExternalInput").ap()
    invsel39_in = nc.dram_tensor("invsel39", [128, 1], f32, kind="ExternalInput").ap()
    out_dram = nc.dram_tensor("out", [O, MY_N], f32, kind="ExternalOutput").ap()

    RG = [list(range(N_CORES))]

    with tile.TileContext(nc) as tc:
        with (
            tc.tile_pool(name="const", bufs=1) as constp,
            tc.tile_pool(name="ps_work", bufs=2, space="PSUM") as ps_work,
            tc.tile_pool(name="ps_s", bufs=2, space="PSUM") as ps_s,
            tc.tile_pool(name="ps_f", bufs=2, space="PSUM") as ps_f,
            tc.tile_pool(name="ps_vals", bufs=1, space="PSUM") as ps_vals,
            tc.tile_pool(name="dram", bufs=1, space="DRAM") as dram,
        ):
            pro = tc.alloc_tile_pool(name="pro", bufs=3)
            # ---- DRAM scratch ------------------------------------------
            d_in = [dram.tile([128, 32], f32, tag=f"din{s}", name=f"din{s}")
                    for s in range(N_STRIPES)]
            d_out = [dram.tile([128 * N_CORES, 32], f32, tag=f"dout{s}",
                               name=f"dout{s}") for s in range(N_STRIPES)]
            st_in = dram.tile([1, 32], f32, tag="stin", name="stin")
            st_out = dram.tile([1, 32], f32, tag="stout", name="stout")
            dum_in = dram.tile([1, 16], f32, tag="dumin", name="dumin")
            dum_out = dram.tile([1, 16], f32, tag="dumout", name="dumout")
            arow_dram = dram.tile([H, MY_N], f32, tag="arowd", name="arowd")

            # ---- constants ---------------------------------------------
            identf = constp.tile([128, 128], f32, tag="identf", name="identf")
            nc.sync.dma_start(identf[:], identf_in)
            ones_col = constp.tile([128, 1], f32, tag="ones_col", name="ones_col")
            nc.vector.memset(ones_col[:], 1.0)
            ones_row = constp.tile([1, 512], f32, tag="ones_row", name="ones_row")
            nc.vector.memset(ones_row[:], 1.0)
            sel39 = constp.tile([128, 1], f32, tag="sel39", name="sel39")
            nc.sync.dma_start(sel39[:], sel39_in)
            invsel39 = constp.tile([128, 1], f32, tag="invsel39", name="invsel39")
            nc.sync.dma_start(invsel39[:], invsel39_in)

            # warm up the collective stack early
            zr = constp.tile([1, 16], f32, tag="zr", name="zr")
            nc.vector.memset(zr[:], 0.0)
            nc.sync.dma_start(dum_in[:], zr[:])
            nc.gpsimd.collective_compute(
                "AllReduce", Alu.add, replica_groups=RG,
                ins=[dum_in.opt()], outs=[dum_out.opt()])

            # ---- resident inputs ---------------------------------------
            inputT = [constp.tile([128, N], bf16, tag=f"inT{fc}", name=f"inT{fc}")
                      for fc in range(FC)]
            for fc in range(FC):
                nc.sync.dma_start(inputT[fc][:], inputT_in[fc])
            rnaT = [constp.tile([128, MY_N], bf16, tag=f"rnaT{fc}", name=f"rnaT{fc}")
                    for fc in range(FC)]
            disT = [constp.tile([128, MY_N], bf16, tag=f"disT{fc}", name=f"disT{fc}")
                    for fc in range(FC)]
            rowsT = [constp.tile([128, MY_N], bf16, tag=f"rowsT{fc}", name=f"rowsT{fc}")
                     for fc in range(FC)]
            for fc in range(FC):
                nc.sync.dma_start(rnaT[fc][:], rowsT_rna_in[fc])
                nc.sync.dma_start(disT[fc][:], rowsT_dis_in[fc])
                nc.vector.tensor_add(rowsT[fc][:], rnaT[fc][:], disT[fc][:])

            wrT = [constp.tile([128, 128], bf16, tag=f"wrT{fc}", name=f"wrT{fc}")
                   for fc in range(FC)]
            for fc in range(FC):
                nc.sync.dma_start(wrT[fc][:], wrT_in[fc])
            projc = {}
            for t in range(2):
                for fc in range(FC):
                    pt = constp.tile([128, H * O], bf16, tag=f"pj{t}{fc}",
                                     name=f"pj{t}{fc}")
                    nc.sync.dma_start(pt[:], projcat_in[t, fc])
                    projc[(t, fc)] = pt

            # ---- q_rhs[t,fc] = [128f, 8] (cols 0-3 src h, 4-7 tgt h) ----
            q_rhs = {(t, fc): constp.tile([128, 8], bf16, tag=f"q{t}{fc}",
                                          name=f"q{t}{fc}")
                     for t in range(2) for fc in range(FC)}
            for si in range(2):
                for h in range(H):
                    srow0 = pro.tile([1, 128], f32, tag="srow0", name="srow0",
                                     bufs=2)
                    nc.sync.dma_start(srow0[:], scores_in[si * 4 + h:si * 4 + h + 1, :])
                    wb = pro.tile([128, 128], f32, tag="wb", name="wb", bufs=2)
                    nc.gpsimd.partition_broadcast(wb[:], srow0[:])
                    for t in range(2):
                        for fc in range(FC):
                            qcol = pro.tile([128, 1], f32, tag="qcol", name="qcol",
                                            bufs=2)
                            qscr = pro.tile([128, O], f32, tag="qscr", name="qscr",
                                            bufs=2)
                            nc.vector.scalar_tensor_tensor(
                                qscr[:], projc[(t, fc)][:, h * O:(h + 1) * O], 1.0,
                                wb[:], op0=Alu.mult, op1=Alu.mult,
                                accum_out=qcol[:])
                            nc.vector.tensor_copy(
                                q_rhs[(t, fc)][:, si * 4 + h:si * 4 + h + 1],
                                qcol[:])

            # ---- s for all chunks: s_big [128, 64*8] f32 ----------------
            def chunk_types(ch):
                if ch < SPLIT_CH:
                    return [0]
                if ch > SPLIT_CH:
                    return [1]
                return [0, 1]

            s_big = constp.tile([128, NCH * 8], f32, tag="sbig", name="sbig")
            for ch in range(NCH):
                tps = chunk_types(ch)
                res = {}
                for t in tps:
                    ps_sc = ps_s.tile([128, 8], f32, tag="small", name="pssc")
                    for fc in range(FC):
                        nc.tensor.matmul(
                            ps_sc[:], inputT[fc][:, ch * 128:(ch + 1) * 128],
                            q_rhs[(t, fc)], start=(fc == 0), stop=(fc == FC - 1))
                    if len(tps) == 1:
                        nc.vector.tensor_copy(s_big[:, ch * 8:ch * 8 + 8], ps_sc[:])
                    else:
                        tmp = pro.tile([128, 8], f32, tag="stmp", name="stmp",
                                       bufs=3)
                        nc.vector.tensor_copy(tmp[:], ps_sc[:])
                        res[t] = tmp
                if len(tps) == 2:
                    t1 = pro.tile([128, 8], f32, tag="sbl", name="sbl", bufs=2)
                    nc.vector.tensor_scalar_mul(t1[:], res[1][:], invsel39[:])
                    nc.vector.scalar_tensor_tensor(
                        s_big[:, ch * 8:ch * 8 + 8], res[0][:], sel39[:], t1[:],
                        op0=Alu.mult, op1=Alu.add)

            # ---- s_src for my rows -> A_bcast[h] [128, MY_N] bf16 -------
            for ic in range(MY_N // 128):
                ps_sr = ps_s.tile([128, 8], f32, tag="small", name="pssr")
                k = 0
                for Tt in (rnaT, disT):
                    for fc in range(FC):
                        nc.tensor.matmul(ps_sr[:], Tt[fc][:, ic * 128:(ic + 1) * 128],
                                         q_rhs[(0 if Tt is rnaT else 1, fc)],
                                         start=(k == 0), stop=(k == 3))
                        k += 1
                srow = pro.tile([128, 8], f32, tag="srow", name="srow", bufs=2)
                nc.vector.tensor_copy(srow[:], ps_sr[:])
                tpsm = ps_work.tile([128, 128], f32, tag="tp", name="tps")
                nc.tensor.transpose(tpsm[0:8, :], srow[:], identf[:])
                srT = pro.tile([8, 128], f32, tag="srT", name="srT", bufs=2)
                nc.vector.tensor_copy(srT[:], tpsm[0:8, :])
                for h in range(H):
                    nc.sync.dma_start(arow_dram[h, ic * 128:(ic + 1) * 128],
                                      srT[h:h + 1, :])
            A_bcast = []
            for h in range(H):
                af = pro.tile([128, MY_N], f32, tag="af", name="af", bufs=2)
                nc.sync.dma_start(af[:], arow_dram[h:h + 1, :].partition_broadcast(128))
                ab = constp.tile([128, MY_N], bf16, tag=f"ab{h}", name=f"ab{h}")
                nc.vector.tensor_copy(ab[:], af[:])
                A_bcast.append(ab)

            # ---- main loop over j-stripes ------------------------------
            pro.release()
            mtp = tc.alloc_tile_pool(name="mtp", bufs=10)
            zp = tc.alloc_tile_pool(name="zp", bufs=2)
            yp = tc.alloc_tile_pool(name="yp", bufs=2)
            ep = tc.alloc_tile_pool(name="ep", bufs=8)
            fsp = tc.alloc_tile_pool(name="fsp", bufs=4)
            gp4 = tc.alloc_tile_pool(name="gp4", bufs=3)
            dpool = tc.alloc_tile_pool(name="dpool", bufs=3)
            vals_ps = ps_vals.tile([128, MY_N], f32, tag="big", name="vals")

            for s in range(N_STRIPES):
                # transposed 0/1 mask tiles for this stripe (plain loads)
                mts = []
                for jt in range(JT):
                    mt = mtp.tile([128, MY_N], bf16, tag="mt", name="mt")
                    nc.sync.dma_start(
                        mt[:], maskT[(s * JT + jt) * 128:(s * JT + jt + 1) * 128, :])
                    mts.append(mt)

                # feats for this stripe's chunks (JIT from resident inputT)
                fsb = []
                for jt in range(JT):
                    ch = s * JT + jt
                    tps = chunk_types(ch)
                    res = {}
                    for t in tps:
                        psf = ps_f.tile([128, H * O], f32, tag="psf", name="psf")
                        for fc in range(FC):
                            nc.tensor.matmul(psf[:], inputT[fc][:, ch * 128:(ch + 1) * 128],
                                             projc[(t, fc)][:],
                                             start=(fc == 0), stop=(fc == FC - 1))
                        if len(tps) == 1:
                            fs = fsp.tile([128, H * O], bf16, tag="fs", name="fs")
                            nc.vector.tensor_copy(fs[:], psf[:])
                        else:
                            tmp = fsp.tile([128, H * O], bf16, tag="fbl", name="fbl",
                                           bufs=2)
                            nc.vector.tensor_copy(tmp[:], psf[:])
                            res[t] = tmp
                    if len(tps) == 2:
                        fs = fsp.tile([128, H * O], bf16, tag="fs", name="fs")
                        t1 = fsp.tile([128, H * O], bf16, tag="fbl2", name="fbl2",
                                      bufs=2)
                        nc.vector.tensor_scalar_mul(t1[:], res[1][:], invsel39[:])
                        nc.vector.scalar_tensor_tensor(
                            fs[:], res[0][:], sel39[:], t1[:],
                            op0=Alu.mult, op1=Alu.add)
                    fsb.append(fs)

                # elementwise chain per j-tile (4 heads batched in free dim)
                # z = mask_add + b_j + a_i (STT); lrelu (ACT batched / DVE);
                # e = exp (ACT, accum -> d partial)
                d_all = dpool.tile([128, 32], f32, tag="dall", name="dall")
                ebs = []
                for jt in range(JT):
                    ch = s * JT + jt
                    zb = zp.tile([128, H * HQ], bf16, tag="zb", name="zb")
                    for h in range(H):
                        nc.vector.scalar_tensor_tensor(
                            zb[:, h * HQ:(h + 1) * HQ], mts[jt][:],
                            s_big[:, ch * 8 + 4 + h:ch * 8 + 4 + h + 1],
                            A_bcast[h][:], op0=Alu.add, op1=Alu.add)
                    yb = yp.tile([128, H * HQ], bf16, tag="yb", name="yb")
                    if (s * JT + jt) in DVE_LRELU:
                        for h in range(H):
                            sl = slice(h * HQ, (h + 1) * HQ)
                            nc.vector.scalar_tensor_tensor(
                                yb[:, sl], zb[:, sl], SLOPE, zb[:, sl],
                                op0=Alu.mult, op1=Alu.max)
                    else:
                        nc.scalar.activation(yb[:], zb[:], Act.Prelu, alpha=SLOPE)
                    eb = ep.tile([128, H * HQ], bf16, tag="eb", name="eb")
                    for h in range(H):
                        sl = slice(h * HQ, (h + 1) * HQ)
                        nc.scalar.activation(
                            eb[:, sl], yb[:, sl], Act.Exp,
                            accum_out=d_all[:, h * 8 + jt:h * 8 + jt + 1])
                    ebs.append(eb)

                # complete d across cores (partial sums over i-rows)
                nc.sync.dma_start(d_in[s][:], d_all[:])
                nc.gpsimd.collective_compute(
                    "AllGather", Alu.bypass, replica_groups=RG,
                    ins=[d_in[s].opt()], outs=[d_out[s].opt()])
                dg = dpool.tile([128, 256], f32, tag="dg", name="dg")
                for r in range(N_CORES):
                    nc.sync.dma_start(dg[:, r * 32:(r + 1) * 32],
                                      d_out[s][r * 128:(r + 1) * 128, :])
                d_sum = dpool.tile([128, 32], f32, tag="dsum", name="dsum")
                nc.vector.tensor_add(d_sum[:], dg[:, 0:32], dg[:, 32:64])
                for r in range(2, N_CORES):
                    nc.vector.tensor_add(d_sum[:], d_sum[:], dg[:, r * 32:(r + 1) * 32])
                dinv = dpool.tile([128, 32], f32, tag="dinv", name="dinv")
                nc.vector.reciprocal(dinv[:], d_sum[:])

                # g = feats/d (GpSimd) ; vals^T += g^T-contract-e
                for jt in range(JT):
                    g4 = gp4.tile([128, H * O], bf16, tag="g4", name="g4")
                    for h in range(H):
                        nc.vector.tensor_scalar_mul(
                            g4[:, h * O:(h + 1) * O], fsb[jt][:, h * O:(h + 1) * O],
                            dinv[:, h * 8 + jt:h * 8 + jt + 1])
                    eb = ebs[jt]
                    for h in range(H):
                        first = (s == 0) and jt == 0 and h == 0
                        last = (s == N_STRIPES - 1) and jt == JT - 1 and h == H - 1
                        nc.tensor.matmul(vals_ps[:, 0:512], g4[:, h * O:(h + 1) * O],
                                         eb[:, h * HQ:h * HQ + 512],
                                         start=first, stop=last)
                        nc.tensor.matmul(vals_ps[:, 512:1024], g4[:, h * O:(h + 1) * O],
                                         eb[:, h * HQ + 512:(h + 1) * HQ],
                                         start=first, stop=last)

            # ---- tail: instance norm + residual + elu ------------------
            dpool.release()
            gp4.release()
            fsp.release()
            ep.release()
            yp.release()
            zp.release()
            mtp.release()
            tailp = tc.alloc_tile_pool(name="tail", bufs=1)
            vs = tailp.tile([128, MY_N], f32, tag="vs", name="vs")
            srow1 = tailp.tile([128, 1], f32, tag="srow1", name="srow1")
            nc.scalar.activation(vs[:], vals_ps[:], Act.Copy, scale=0.25,
                                 accum_out=srow1[:])
            vsq = tailp.tile([128, MY_N], f32, tag="vsq", name="vsq")
            srow2 = tailp.tile([128, 1], f32, tag="srow2", name="srow2")
            nc.scalar.activation(vsq[:], vs[:], Act.Square, accum_out=srow2[:])

            ps1 = ps_s.tile([1, 1], f32, tag="small", name="ps1")
            nc.tensor.matmul(ps1[:], srow1[:], ones_col[:])
            ps2 = ps_s.tile([1, 1], f32, tag="small", name="ps2")
            nc.tensor.matmul(ps2[:], srow2[:], ones_col[:])
            stv = tailp.tile([1, 32], f32, tag="stv", name="stv")
            nc.vector.memset(stv[:], 0.0)
            nc.vector.tensor_copy(stv[0:1, 0:1], ps1[:])
            nc.vector.tensor_copy(stv[0:1, 16:17], ps2[:])
            nc.sync.dma_start(st_in[:], stv[:])
            nc.gpsimd.collective_compute(
                "AllReduce", Alu.add, replica_groups=RG,
                ins=[st_in.opt()], outs=[st_out.opt()])
            str_ = tailp.tile([1, 32], f32, tag="str", name="str")
            nc.sync.dma_start(str_[:], st_out[:])

            c = 1.0 / float(N * O)
            mu = tailp.tile([1, 1], f32, tag="mu", name="mu")
            nc.vector.tensor_scalar_mul(mu[:], str_[0:1, 0:1], c)
            m2 = tailp.tile([1, 1], f32, tag="m2", name="m2")
            nc.vector.tensor_scalar_mul(m2[:], str_[0:1, 16:17], c)
            mu2 = tailp.tile([1, 1], f32, tag="mu2", name="mu2")
            nc.vector.tensor_mul(mu2[:], mu[:], mu[:])
            var = tailp.tile([1, 1], f32, tag="var", name="var")
            nc.vector.tensor_sub(var[:], m2[:], mu2[:])
            vpe = tailp.tile([1, 1], f32, tag="vpe", name="vpe")
            nc.vector.tensor_scalar_add(vpe[:], var[:], EPS)
            sd = tailp.tile([1, 1], f32, tag="sd", name="sd")
            nc.scalar.activation(sd[:], vpe[:], Act.Sqrt)
            rstd = tailp.tile([1, 1], f32, tag="rstd", name="rstd")
            nc.vector.reciprocal(rstd[:], sd[:])
            negmurs = tailp.tile([1, 1], f32, tag="negmurs", name="negmurs")
            nc.vector.tensor_mul(negmurs[:], mu[:], rstd[:])
            nc.vector.tensor_scalar_mul(negmurs[:], negmurs[:], -1.0)

            a_col = tailp.tile([128, 1], f32, tag="acol", name="acol")
            nc.gpsimd.partition_broadcast(a_col[:], rstd[:])
            b_row = tailp.tile([1, 128], f32, tag="brow", name="brow")
            nc.scalar.activation(b_row[:], ones_row[0:1, 0:128], Act.Copy,
                                 scale=negmurs[:])

            r_ps = ps_vals.tile([128, MY_N], f32, tag="big", name="resid")
            for half in range(2):
                sl = slice(half * 512, (half + 1) * 512)
                for fc in range(FC):
                    nc.tensor.matmul(r_ps[:, sl], wrT[fc][:], rowsT[fc][:, sl],
                                     start=(fc == 0), stop=False)
                nc.tensor.matmul(r_ps[:, sl], b_row[:], ones_row[:],
                                 start=False, stop=True)

            pre = tailp.tile([128, MY_N], f32, tag="pre", name="pre")
            nc.vector.scalar_tensor_tensor(pre[:], vs[:], a_col[:], r_ps[:],
                                           op0=Alu.mult, op1=Alu.add)
            negp = tailp.tile([128, MY_N], f32, tag="negp", name="negp")
            nc.vector.tensor_scalar_min(negp[:], pre[:], 0.0)
            w = tailp.tile([128, MY_N], f32, tag="w", name="w")
            nc.scalar.activation(w[:], negp[:], Act.Exp)
            r1 = tailp.tile([128, MY_N], f32, tag="r1", name="r1")
            nc.vector.tensor_scalar_max(r1[:], pre[:], 0.0)
            outt = tailp.tile([128, MY_N], f32, tag="outt", name="outt")
            nc.vector.scalar_tensor_tensor(outt[:], w[:], -1.0, r1[:],
                                           op0=Alu.add, op1=Alu.add)
            nc.sync.dma_start(out_dram, outt[:])
            tailp.release()

    nc.compile()
    return nc


def _get_nc():
    if "nc" not in _cached:
        _cached["nc"] = _build()
    return _cached["nc"]


def kernel(input_mat, connectivity_mask, proj_rna, proj_dis, score_src,
           score_tgt, residual_w):
    from concourse.bass_utils import run_bass_kernel_spmd
    from ml_dtypes import bfloat16

    nc = _get_nc()
    input_mat = np.asarray(input_mat, np.float32)
    connectivity_mask = np.asarray(connectivity_mask, np.float32)
    proj_rna = np.asarray(proj_rna, np.float32)
    proj_dis = np.asarray(proj_dis, np.float32)
    score_src = np.asarray(score_src, np.float32)
    score_tgt = np.asarray(score_tgt, np.float32)
    residual_w = np.asarray(residual_w, np.float32)

    ident = np.eye(128, dtype=np.float32)
    sel39 = (np.arange(128) < SPLIT_ROW).astype(np.float32)[:, None]

    # shared (replicated) prepped tensors
    inputT_b = np.ascontiguousarray(input_mat.T.astype(bfloat16)).reshape(
        FC, 128, N)
    projcat = np.empty((2, FC, 128, H * O), np.float32)
    for t, pj in enumerate((proj_rna, proj_dis)):
        for fc in range(FC):
            for h in range(H):
                projcat[t, fc, :, h * O:(h + 1) * O] = pj[h, fc * 128:(fc + 1) * 128, :]
    projcat_b = projcat.astype(bfloat16)
    scores = np.concatenate([score_src[:, :, 0], score_tgt[:, :, 0]],
                            axis=0).astype(np.float32)  # [8, 128]
    wrT = np.empty((FC, 128, 128), np.float32)
    for fc in range(FC):
        wrT[fc] = residual_w[:, fc * 128:(fc + 1) * 128].T
    wrT_b = wrT.astype(bfloat16)

    rna_mask = (np.arange(N) < N_RNA).astype(np.float32)[:, None]
    in_rna_full = input_mat * rna_mask
    in_dis_full = input_mat * (1.0 - rna_mask)

    in_maps = []
    for k in range(N_CORES):
        r0, r1 = k * MY_N, (k + 1) * MY_N
        maskT_k = np.ascontiguousarray(
            connectivity_mask[r0:r1, :].astype(bfloat16).T)
        rowsT_rna = np.ascontiguousarray(
            in_rna_full[r0:r1].T.astype(bfloat16)).reshape(FC, 128, MY_N)
        rowsT_dis = np.ascontiguousarray(
            in_dis_full[r0:r1].T.astype(bfloat16)).reshape(FC, 128, MY_N)
        in_maps.append({
            "maskT": maskT_k,
            "inputT": inputT_b,
            "rowsT_rna": rowsT_rna,
            "rowsT_dis": rowsT_dis,
            "projcat": projcat_b,
            "scores": scores,
            "wrT": wrT_b,
            "identf": ident,
            "sel39": sel39,
            "invsel39": 1.0 - sel39,
        })

    res = run_bass_kernel_spmd(nc, in_maps, core_ids=list(range(N_CORES)))
    _cached["last_result"] = res
    out = np.empty((N, O), np.float32)
    for k in range(N_CORES):
        out[k * MY_N:(k + 1) * MY_N, :] = res.results[k]["out"].T
    return out


# revision 23
# speedup vs baseline: 2.3393x; 1.1362x over previous
"""Trainium2 Bass kernel for nn_HGraphAttentionLayer (GAT-style layer, 8 NeuronCores).

Math (reference):
  feats[h,n,o]  = concat(input[:5000] @ proj_rna[h], input[5000:] @ proj_dis[h])
  s_src[h,n]    = feats[h,n,:] @ score_src[h];  s_tgt likewise
  attn[h,i,j]   = softmax_over_i( mask[i,j] + leaky_relu(s_src[h,i]+s_tgt[h,j], 0.2) )
  vals[i,o]     = mean_h( sum_j attn[h,i,j] * feats[h,j,o] )
  out           = elu( instancenorm(vals) + input @ residual_w.T )

Sharding: each core owns N/8 = 1024 query rows (i). Softmax reduces over i,
so partial column sums d[h,j] are AllGathered per j-stripe.

Key design (v2):
 - mask passed from host as a pre-transposed {0,1} bf16 indicator M^T[j,i];
   e = M * exp(lrelu(a_i+b_j)) so the mask enters via one 2x-mode
   tensor_tensor_reduce (which also produces d via accum) instead of a
   1x-mode 3-input add chain.
 - z = a+b is a 4x-mode tensor_scalar (per-partition scalar b_j).
 - lrelu runs on ACT (Prelu, batched [128,4096] over 4 heads) for most
   j-tiles and on DVE (2-op mul/max) for a tunable fraction, balancing
   the two engines. exp is one batched ACT op per j-tile.
 - feats computed just-in-time per stripe from SBUF-resident inputT
   (no DRAM spills); input/proj/residual weights pre-cast + pre-transposed
   on host.
 - g = feats/d runs on the otherwise-idle GpSimd engine.
"""
import numpy as np

N, F, H, O = 8192, 256, 4, 128
N_CORES = 8
MY_N = N // N_CORES          # 1024 rows per core
N_RNA = 5000
SLOPE = 0.2
EPS = 1e-5
N_STRIPES = 8
JT = 8                       # j-tiles (128 j each) per stripe
NCH = N // 128               # 64 node chunks
FC = F // 128                # 2 f chunks
SPLIT_CH = N_RNA // 128      # chunk 39 contains the rna/dis boundary
SPLIT_ROW = N_RNA - SPLIT_CH * 128  # row 8 within chunk 39
HQ = MY_N                    # head-quadrant stride in the batched tiles

# j-tile slots (s*8+jt) where lrelu runs on DVE instead of ACT (balance knob)
DVE_LRELU = {i for i in range(64) if i % 4 == 1}  # 16 of 64 slots




_cached = {}


def _build():
    import concourse.bass as bass
    import concourse.bacc as bacc
    import concourse.mybir as mybir
    import concourse.tile as tile

    f32 = mybir.dt.float32
    bf16 = mybir.dt.bfloat16
    Alu = mybir.AluOpType
    Act = mybir.ActivationFunctionType

    nc = bacc.Bacc("TRN2", target_bir_lowering=False, debug=False,
                   enable_asserts=False, num_devices=N_CORES)

    # ---- I/O -----------------------------------------------------------
    maskT = nc.dram_tensor("maskT", [N, MY_N], bf16, kind="ExternalInput").ap()
    inputT_in = nc.dram_tensor("inputT", [FC, 128, N], bf16, kind="ExternalInput").ap()
    rowsT_rna_in = nc.dram_tensor("rowsT_rna", [FC, 128, MY_N], bf16,
                                  kind="ExternalInput").ap()
    rowsT_dis_in = nc.dram_tensor("rowsT_dis", [FC, 128, MY_N], bf16,
                                  kind="ExternalInput").ap()
    projcat_in = nc.dram_tensor("projcat", [2, FC, 128, H * O], bf16,
                                kind="ExternalInput").ap()
    scores_in = nc.dram_tensor("scores", [8, 128], f32, kind="ExternalInput").ap()
    wrT_in = nc.dram_tensor("wrT", [FC, 128, 128], bf16, kind="ExternalInput").ap()
    identf_in = nc.dram_tensor("identf", [128, 128], f32, kind="ExternalInput").ap()
    sel39_in = nc.dram_tensor("sel39", [128, 1], f32, kind="ExternalInput").ap()
    invsel39_in = nc.dram_tensor("invsel39", [128, 1], f32, kind="ExternalInput").ap()
    out_dram = nc.dram_tensor("out", [O, MY_N], f32, kind="ExternalOutput").ap()

    RG = [list(range(N_CORES))]

    with tile.TileContext(nc) as tc:
        with (
            tc.tile_pool(name="const", bufs=1) as constp,
            tc.tile_pool(name="ps_work", bufs=2, space="PSUM") as ps_work,
            tc.tile_pool(name="ps_s", bufs=2, space="PSUM") as ps_s,
            tc.tile_pool(name="ps_f", bufs=2, space="PSUM") as ps_f,
            tc.tile_pool(name="ps_vals", bufs=1, space="PSUM") as ps_vals,
            tc.tile_pool(name="dram", bufs=1, space="DRAM") as dram,
        ):
            pro = tc.alloc_tile_pool(name="pro", bufs=3)
            # ---- DRAM scratch ------------------------------------------
            d_in = [dram.tile([128, 32], f32, tag=f"din{s}", name=f"din{s}")
                    for s in range(N_STRIPES)]
            d_out = [dram.tile([128 * N_CORES, 32], f32, tag=f"dout{s}",
                               name=f"dout{s}") for s in range(N_STRIPES)]
            st_in = dram.tile([1, 32], f32, tag="stin", name="stin")
            st_out = dram.tile([1, 32], f32, tag="stout", name="stout")
            dum_in = dram.tile([1, 16], f32, tag="dumin", name="dumin")
            dum_out = dram.tile([1, 16], f32, tag="dumout", name="dumout")
            arow_dram = dram.tile([H, MY_N], f32, tag="arowd", name="arowd")
            feats_dram = dram.tile([NCH, 128, H * O], bf16, tag="featsd",
                                   name="featsd")

            # ---- constants ---------------------------------------------
            identf = pro.tile([128, 128], f32, tag="identf", name="identf", bufs=1)
            nc.sync.dma_start(identf[:], identf_in)
            ones_col = constp.tile([128, 1], f32, tag="ones_col", name="ones_col")
            nc.vector.memset(ones_col[:], 1.0)
            ones_row = constp.tile([1, 512], f32, tag="ones_row", name="ones_row")
            nc.vector.memset(ones_row[:], 1.0)
            sel39 = constp.tile([128, 1], f32, tag="sel39", name="sel39")
            nc.sync.dma_start(sel39[:], sel39_in)
            invsel39 = constp.tile([128, 1], f32, tag="invsel39", name="invsel39")
            nc.sync.dma_start(invsel39[:], invsel39_in)

            # warm up the collective stack early
            zr = constp.tile([1, 16], f32, tag="zr", name="zr")
            nc.vector.memset(zr[:], 0.0)
            nc.sync.dma_start(dum_in[:], zr[:])
            nc.gpsimd.collective_compute(
                "AllReduce", Alu.add, replica_groups=RG,
                ins=[dum_in.opt()], outs=[dum_out.opt()])

            # ---- resident inputs ---------------------------------------
            inputT = [pro.tile([128, N], bf16, tag=f"inT{fc}", name=f"inT{fc}", bufs=1)
                      for fc in range(FC)]
            for fc in range(FC):
                nc.sync.dma_start(inputT[fc][:], inputT_in[fc])
            rnaT = [pro.tile([128, MY_N], bf16, tag=f"rnaT{fc}", name=f"rnaT{fc}", bufs=1)
                    for fc in range(FC)]
            disT = [pro.tile([128, MY_N], bf16, tag=f"disT{fc}", name=f"disT{fc}", bufs=1)
                    for fc in range(FC)]
            rowsT = [constp.tile([128, MY_N], bf16, tag=f"rowsT{fc}", name=f"rowsT{fc}")
                     for fc in range(FC)]
            for fc in range(FC):
                nc.sync.dma_start(rnaT[fc][:], rowsT_rna_in[fc])
                nc.sync.dma_start(disT[fc][:], rowsT_dis_in[fc])
                nc.vector.tensor_add(rowsT[fc][:], rnaT[fc][:], disT[fc][:])

            wrT = [constp.tile([128, 128], bf16, tag=f"wrT{fc}", name=f"wrT{fc}")
                   for fc in range(FC)]
            for fc in range(FC):
                nc.sync.dma_start(wrT[fc][:], wrT_in[fc])
            projc = {}
            for t in range(2):
                for fc in range(FC):
                    pt = pro.tile([128, H * O], bf16, tag=f"pj{t}{fc}",
                                  name=f"pj{t}{fc}", bufs=1)
                    nc.sync.dma_start(pt[:], projcat_in[t, fc])
                    projc[(t, fc)] = pt

            # ---- q_rhs[t,fc] = [128f, 8] (cols 0-3 src h, 4-7 tgt h) ----
            q_rhs = {(t, fc): pro.tile([128, 8], bf16, tag=f"q{t}{fc}",
                                       name=f"q{t}{fc}", bufs=1)
                     for t in range(2) for fc in range(FC)}
            for si in range(2):
                for h in range(H):
                    srow0 = pro.tile([1, 128], f32, tag="srow0", name="srow0",
                                     bufs=2)
                    nc.sync.dma_start(srow0[:], scores_in[si * 4 + h:si * 4 + h + 1, :])
                    wb = pro.tile([128, 128], f32, tag="wb", name="wb", bufs=2)
                    nc.gpsimd.partition_broadcast(wb[:], srow0[:])
                    for t in range(2):
                        for fc in range(FC):
                            qcol = pro.tile([128, 1], f32, tag="qcol", name="qcol",
                                            bufs=2)
                            qscr = pro.tile([128, O], f32, tag="qscr", name="qscr",
                                            bufs=2)
                            nc.vector.scalar_tensor_tensor(
                                qscr[:], projc[(t, fc)][:, h * O:(h + 1) * O], 1.0,
                                wb[:], op0=Alu.mult, op1=Alu.mult,
                                accum_out=qcol[:])
                            nc.vector.tensor_copy(
                                q_rhs[(t, fc)][:, si * 4 + h:si * 4 + h + 1],
                                qcol[:])

            # ---- s for all chunks: s_big [128, 64*8] f32 ----------------
            def chunk_types(ch):
                if ch < SPLIT_CH:
                    return [0]
                if ch > SPLIT_CH:
                    return [1]
                return [0, 1]

            s_big = constp.tile([128, NCH * 8], f32, tag="sbig", name="sbig")
            for ch in range(NCH):
                tps = chunk_types(ch)
                res = {}
                for t in tps:
                    ps_sc = ps_s.tile([128, 8], f32, tag="small", name="pssc")
                    for fc in range(FC):
                        nc.tensor.matmul(
                            ps_sc[:], inputT[fc][:, ch * 128:(ch + 1) * 128],
                            q_rhs[(t, fc)], start=(fc == 0), stop=(fc == FC - 1))
                    if len(tps) == 1:
                        nc.vector.tensor_copy(s_big[:, ch * 8:ch * 8 + 8], ps_sc[:])
                    else:
                        tmp = pro.tile([128, 8], f32, tag="stmp", name="stmp",
                                       bufs=3)
                        nc.vector.tensor_copy(tmp[:], ps_sc[:])
                        res[t] = tmp
                if len(tps) == 2:
                    t1 = pro.tile([128, 8], f32, tag="sbl", name="sbl", bufs=2)
                    nc.vector.tensor_scalar_mul(t1[:], res[1][:], invsel39[:])
                    nc.vector.scalar_tensor_tensor(
                        s_big[:, ch * 8:ch * 8 + 8], res[0][:], sel39[:], t1[:],
                        op0=Alu.mult, op1=Alu.add)

            # ---- s_src for my rows -> A_bcast[h] [128, MY_N] bf16 -------
            for ic in range(MY_N // 128):
                ps_sr = ps_s.tile([128, 8], f32, tag="small", name="pssr")
                k = 0
                for Tt in (rnaT, disT):
                    for fc in range(FC):
                        nc.tensor.matmul(ps_sr[:], Tt[fc][:, ic * 128:(ic + 1) * 128],
                                         q_rhs[(0 if Tt is rnaT else 1, fc)],
                                         start=(k == 0), stop=(k == 3))
                        k += 1
                srow = pro.tile([128, 8], f32, tag="srow", name="srow", bufs=2)
                nc.vector.tensor_copy(srow[:], ps_sr[:])
                tpsm = ps_work.tile([128, 128], f32, tag="tp", name="tps")
                nc.tensor.transpose(tpsm[0:8, :], srow[:], identf[:])
                srT = pro.tile([8, 128], f32, tag="srT", name="srT", bufs=2)
                nc.vector.tensor_copy(srT[:], tpsm[0:8, :])
                for h in range(H):
                    nc.sync.dma_start(arow_dram[h, ic * 128:(ic + 1) * 128],
                                      srT[h:h + 1, :])
            A_bcast = []
            for h in range(H):
                af = pro.tile([128, MY_N], f32, tag="af", name="af", bufs=2)
                nc.sync.dma_start(af[:], arow_dram[h:h + 1, :].partition_broadcast(128))
                ab = constp.tile([128, MY_N], bf16, tag=f"ab{h}", name=f"ab{h}")
                nc.vector.tensor_copy(ab[:], af[:])
                A_bcast.append(ab)

            # ---- feats for all chunks -> DRAM spill --------------------
            for ch in range(NCH):
                tps = chunk_types(ch)
                res = {}
                for t in tps:
                    psf = ps_f.tile([128, H * O], f32, tag="psf", name="psf")
                    for fc in range(FC):
                        nc.tensor.matmul(psf[:], inputT[fc][:, ch * 128:(ch + 1) * 128],
                                         projc[(t, fc)][:],
                                         start=(fc == 0), stop=(fc == FC - 1))
                    if len(tps) == 1:
                        fs = pro.tile([128, H * O], bf16, tag="fs", name="fs", bufs=4)
                        nc.vector.tensor_copy(fs[:], psf[:])
                    else:
                        tmp = pro.tile([128, H * O], bf16, tag="fbl", name="fbl",
                                       bufs=2)
                        nc.vector.tensor_copy(tmp[:], psf[:])
                        res[t] = tmp
                if len(tps) == 2:
                    fs = pro.tile([128, H * O], bf16, tag="fs", name="fs", bufs=4)
                    t1 = pro.tile([128, H * O], bf16, tag="fbl2", name="fbl2",
                                  bufs=2)
                    nc.vector.tensor_scalar_mul(t1[:], res[1][:], invsel39[:])
                    nc.vector.scalar_tensor_tensor(
                        fs[:], res[0][:], sel39[:], t1[:],
                        op0=Alu.mult, op1=Alu.add)
                nc.sync.dma_start(feats_dram[ch], fs[:])

            # ---- main loop over j-stripes ------------------------------
            pro.release()
            mtp = tc.alloc_tile_pool(name="mtp", bufs=10)
            zp = tc.alloc_tile_pool(name="zp", bufs=2)
            yp = tc.alloc_tile_pool(name="yp", bufs=2)
            ep = tc.alloc_tile_pool(name="ep", bufs=12)
            fsp = tc.alloc_tile_pool(name="fsp", bufs=16)
            gp4 = tc.alloc_tile_pool(name="gp4", bufs=3)
            dpool = tc.alloc_tile_pool(name="dpool", bufs=3)
            vals_ps = ps_vals.tile([128, MY_N], f32, tag="big", name="vals")

            for s in range(N_STRIPES):
                # transposed additive-mask tiles for this stripe (plain loads)
                mts = []
                for jt in range(JT):
                    mt = mtp.tile([128, MY_N], bf16, tag="mt", name="mt")
                    nc.sync.dma_start(
                        mt[:], maskT[(s * JT + jt) * 128:(s * JT + jt + 1) * 128, :])
                    mts.append(mt)

                # feats for this stripe's chunks (from DRAM spill)
                fsb = []
                for jt in range(JT):
                    fs = fsp.tile([128, H * O], bf16, tag="fs", name="fs")
                    nc.sync.dma_start(fs[:], feats_dram[s * JT + jt])
                    fsb.append(fs)

                # elementwise chain per j-tile (4 heads batched in free dim)
                # z = mask_add + b_j + a_i (STT); lrelu (ACT batched / DVE);
                # e = exp (ACT, accum -> d partial)
                d_all = dpool.tile([128, 32], f32, tag="dall", name="dall")
                ebs = []
                for jt in range(JT):
                    ch = s * JT + jt
                    zb = zp.tile([128, H * HQ], bf16, tag="zb", name="zb")
                    for h in range(H):
                        nc.vector.scalar_tensor_tensor(
                            zb[:, h * HQ:(h + 1) * HQ], mts[jt][:],
                            s_big[:, ch * 8 + 4 + h:ch * 8 + 4 + h + 1],
                            A_bcast[h][:], op0=Alu.add, op1=Alu.add)
                    yb = yp.tile([128, H * HQ], bf16, tag="yb", name="yb")
                    if (s * JT + jt) in DVE_LRELU:
                        for h in range(H):
                            sl = slice(h * HQ, (h + 1) * HQ)
                            nc.vector.scalar_tensor_tensor(
                                yb[:, sl], zb[:, sl], SLOPE, zb[:, sl],
                                op0=Alu.mult, op1=Alu.max)
                    else:
                        nc.scalar.activation(yb[:], zb[:], Act.Prelu, alpha=SLOPE)
                    eb = ep.tile([128, H * HQ], bf16, tag="eb", name="eb")
                    for h in range(H):
                        sl = slice(h * HQ, (h + 1) * HQ)
                        nc.scalar.activation(
                            eb[:, sl], yb[:, sl], Act.Exp,
                            accum_out=d_all[:, h * 8 + jt:h * 8 + jt + 1])
                    ebs.append(eb)

                # complete d across cores (partial sums over i-rows)
                nc.sync.dma_start(d_in[s][:], d_all[:])
                nc.gpsimd.collective_compute(
                    "AllGather", Alu.bypass, replica_groups=RG,
                    ins=[d_in[s].opt()], outs=[d_out[s].opt()])
                dg = dpool.tile([128, 256], f32, tag="dg", name="dg")
                for r in range(N_CORES):
                    nc.sync.dma_start(dg[:, r * 32:(r + 1) * 32],
                                      d_out[s][r * 128:(r + 1) * 128, :])
                d_sum = dpool.tile([128, 32], f32, tag="dsum", name="dsum")
                nc.vector.tensor_add(d_sum[:], dg[:, 0:32], dg[:, 32:64])
                for r in range(2, N_CORES):
                    nc.vector.tensor_add(d_sum[:], d_sum[:], dg[:, r * 32:(r + 1) * 32])
                dinv = dpool.tile([128, 32], f32, tag="dinv", name="dinv")
                nc.vector.reciprocal(dinv[:], d_sum[:])

                # g = feats/d (GpSimd) ; vals^T += g^T-contract-e
                for jt in range(JT):
                    g4 = gp4.tile([128, H * O], bf16, tag="g4", name="g4")
                    for h in range(H):
                        nc.vector.tensor_scalar_mul(
                            g4[:, h * O:(h + 1) * O], fsb[jt][:, h * O:(h + 1) * O],
                            dinv[:, h * 8 + jt:h * 8 + jt + 1])
                    eb = ebs[jt]
                    for h in range(H):
                        first = (s == 0) and jt == 0 and h == 0
                        last = (s == N_STRIPES - 1) and jt == JT - 1 and h == H - 1
                        nc.tensor.matmul(vals_ps[:, 0:512], g4[:, h * O:(h + 1) * O],
                                         eb[:, h * HQ:h * HQ + 512],
                                         start=first, stop=last)
                        nc.tensor.matmul(vals_ps[:, 512:1024], g4[:, h * O:(h + 1) * O],
                                         eb[:, h * HQ + 512:(h + 1) * HQ],
                                         start=first, stop=last)

            # ---- tail: instance norm + residual + elu ------------------
            dpool.release()
            gp4.release()
            fsp.release()
            ep.release()
            yp.release()
            zp.release()
            mtp.release()
            tailp = tc.alloc_tile_pool(name="tail", bufs=1)
            vs = tailp.tile([128, MY_N], f32, tag="vs", name="vs")
            srow1 = tailp.tile([128, 1], f32, tag="srow1", name="srow1")
            nc.scalar.activation(vs[:], vals_ps[:], Act.Copy, scale=0.25,
                                 accum_out=srow1[:])
            vsq = tailp.tile([128, MY_N], f32, tag="vsq", name="vsq")
            srow2 = tailp.tile([128, 1], f32, tag="srow2", name="srow2")
            nc.scalar.activation(vsq[:], vs[:], Act.Square, accum_out=srow2[:])

            ps1 = ps_s.tile([1, 1], f32, tag="small", name="ps1")
            nc.tensor.matmul(ps1[:], srow1[:], ones_col[:])
            ps2 = ps_s.tile([1, 1], f32, tag="small", name="ps2")
            nc.tensor.matmul(ps2[:], srow2[:], ones_col[:])
            stv = tailp.tile([1, 32], f32, tag="stv", name="stv")
            nc.vector.memset(stv[:], 0.0)
            nc.vector.tensor_copy(stv[0:1, 0:1], ps1[:])
            nc.vector.tensor_copy(stv[0:1, 16:17], ps2[:])
            nc.sync.dma_start(st_in[:], stv[:])
            nc.gpsimd.collective_compute(
                "AllReduce", Alu.add, replica_groups=RG,
                ins=[st_in.opt()], outs=[st_out.opt()])
            str_ = tailp.tile([1, 32], f32, tag="str", name="str")
            nc.sync.dma_start(str_[:], st_out[:])

            c = 1.0 / float(N * O)
            mu = tailp.tile([1, 1], f32, tag="mu", name="mu")
            nc.vector.tensor_scalar_mul(mu[:], str_[0:1, 0:1], c)
            m2 = tailp.tile([1, 1], f32, tag="m2", name="m2")
            nc.vector.tensor_scalar_mul(m2[:], str_[0:1, 16:17], c)
            mu2 = tailp.tile([1, 1], f32, tag="mu2", name="mu2")
            nc.vector.tensor_mul(mu2[:], mu[:], mu[:])
            var = tailp.tile([1, 1], f32, tag="var", name="var")
            nc.vector.tensor_sub(var[:], m2[:], mu2[:])
            vpe = tailp.tile([1, 1], f32, tag="vpe", name="vpe")
            nc.vector.tensor_scalar_add(vpe[:], var[:], EPS)
            sd = tailp.tile([1, 1], f32, tag="sd", name="sd")
            nc.scalar.activation(sd[:], vpe[:], Act.Sqrt)
            rstd = tailp.tile([1, 1], f32, tag="rstd", name="rstd")
            nc.vector.reciprocal(rstd[:], sd[:])
            negmurs = tailp.tile([1, 1], f32, tag="negmurs", name="negmurs")
            nc.vector.tensor_mul(negmurs[:], mu[:], rstd[:])
            nc.vector.tensor_scalar_mul(negmurs[:], negmurs[:], -1.0)

            a_col = tailp.tile([128, 1], f32, tag="acol", name="acol")
            nc.gpsimd.partition_broadcast(a_col[:], rstd[:])
            b_row = tailp.tile([1, 128], f32, tag="brow", name="brow")
            nc.scalar.activation(b_row[:], ones_row[0:1, 0:128], Act.Copy,
                                 scale=negmurs[:])

            r_ps = ps_vals.tile([128, MY_N], f32, tag="big", name="resid")
            for half in range(2):
                sl = slice(half * 512, (half + 1) * 512)
                for fc in range(FC):
                    nc.tensor.matmul(r_ps[:, sl], wrT[fc][:], rowsT[fc][:, sl],
                                     start=(fc == 0), stop=False)
                nc.tensor.matmul(r_ps[:, sl], b_row[:], ones_row[:],
                                 start=False, stop=True)

            pre = tailp.tile([128, MY_N], f32, tag="pre", name="pre")
            nc.vector.scalar_tensor_tensor(pre[:], vs[:], a_col[:], r_ps[:],
                                           op0=Alu.mult, op1=Alu.add)
            negp = tailp.tile([128, MY_N], f32, tag="negp", name="negp")
            nc.vector.tensor_scalar_min(negp[:], pre[:], 0.0)
            w = tailp.tile([128, MY_N], f32, tag="w", name="w")
            nc.scalar.activation(w[:], negp[:], Act.Exp)
            r1 = tailp.tile([128, MY_N], f32, tag="r1", name="r1")
            nc.vector.tensor_scalar_max(r1[:], pre[:], 0.0)
            outt = tailp.tile([128, MY_N], f32, tag="outt", name="outt")
            nc.vector.scalar_tensor_tensor(outt[:], w[:], -1.0, r1[:],
                                           op0=Alu.add, op1=Alu.add)
            nc.sync.dma_start(out_dram, outt[:])
            tailp.release()

    nc.compile()
    return nc


def _get_nc():
    if "nc" not in _cached:
        _cached["nc"] = _build()
    return _cached["nc"]


def kernel(input_mat, connectivity_mask, proj_rna, proj_dis, score_src,
           score_tgt, residual_w):
    from concourse.bass_utils import run_bass_kernel_spmd
    from ml_dtypes import bfloat16

    nc = _get_nc()
    input_mat = np.asarray(input_mat, np.float32)
    connectivity_mask = np.asarray(connectivity_mask, np.float32)
    proj_rna = np.asarray(proj_rna, np.float32)
    proj_dis = np.asarray(proj_dis, np.float32)
    score_src = np.asarray(score_src, np.float32)
    score_tgt = np.asarray(score_tgt, np.float32)
    residual_w = np.asarray(residual_w, np.float32)

    ident = np.eye(128, dtype=np.float32)
    sel39 = (np.arange(128) < SPLIT_ROW).astype(np.float32)[:, None]

    # shared (replicated) prepped tensors
    inputT_b = np.ascontiguousarray(input_mat.T.astype(bfloat16)).reshape(
        FC, 128, N)
    projcat = np.empty((2, FC, 128, H * O), np.float32)
    for t, pj in enumerate((proj_rna, proj_dis)):
        for fc in range(FC):
            for h in range(H):
                projcat[t, fc, :, h * O:(h + 1) * O] = pj[h, fc * 128:(fc + 1) * 128, :]
    projcat_b = projcat.astype(bfloat16)
    scores = np.concatenate([score_src[:, :, 0], score_tgt[:, :, 0]],
                            axis=0).astype(np.float32)  # [8, 128]
    wrT = np.empty((FC, 128, 128), np.float32)
    for fc in range(FC):
        wrT[fc] = residual_w[:, fc * 128:(fc + 1) * 128].T
    wrT_b = wrT.astype(bfloat16)

    rna_mask = (np.arange(N) < N_RNA).astype(np.float32)[:, None]
    in_rna_full = input_mat * rna_mask
    in_dis_full = input_mat * (1.0 - rna_mask)

    in_maps = []
    for k in range(N_CORES):
        r0, r1 = k * MY_N, (k + 1) * MY_N
        maskT_k = np.ascontiguousarray(
            connectivity_mask[r0:r1, :].astype(bfloat16).T)
        rowsT_rna = np.ascontiguousarray(
            in_rna_full[r0:r1].T.astype(bfloat16)).reshape(FC, 128, MY_N)
        rowsT_dis = np.ascontiguousarray(
            in_dis_full[r0:r1].T.astype(bfloat16)).reshape(FC, 128, MY_N)
        in_maps.append({
            "maskT": maskT_k,
            "inputT": inputT_b,
            "rowsT_rna": rowsT_rna,
            "rowsT_dis": rowsT_dis,
            "projcat": projcat_b,
            "scores": scores,
            "wrT": wrT_b,
            "identf": ident,
            "sel39": sel39,
            "invsel39": 1.0 - sel39,
        })

    res = run_bass_kernel_spmd(nc, in_maps, core_ids=list(range(N_CORES)))
    _cached["last_result"] = res
    out = np.empty((N, O), np.float32)
    for k in range(N_CORES):
        out[k * MY_N:(k + 1) * MY_N, :] = res.results[k]["out"].T
    return out
